# revision 92
# baseline (speedup 1.0000x reference)
"""Trainium2 Bass kernel for nn_CrossSpaceAttention (batch 8, DIM=128, HEADS=8,
128x128 spatial). Data-parallel over batch: one sample per NeuronCore x8.

v3: 5-pair fp8 DoubleRow convs, host-prepadded chunked input DMAs, fp8
DoubleRow y-conv with exact wrap-correction columns, bf16 attn phase and
bf16 output DMA.

Per-core algorithm:
  qT[n,c] = sum_t x0shift_t(n,:)^T A_t^T   (fp8 DoubleRow: 9 taps packed in
            5 pair-matmuls; outputs land pixel-major = Gram-ready)
  kT likewise; per 128-pixel row chunk, PSUM -> fp8 SBUF qkT8 (pure copy).
  Per row-pair chunk and head-group g: DoubleRow Gram matmuls accumulate
    [q_g^T q_g | q_g^T k_g]  (self block gives norms on its diagonal),
    k_g^T k_g, and ones-column sums Sq, Sk.
  Bias (incl. its uniform interior part; SAME-border deltas are negligible
  for the attention path) is applied algebraically: rank-2 correction
  matmuls  G += bq (x) (Sk + N bk) + Sq (x) bk  close each group exactly.
  attn = softmax(0.25 * G / (|q| |k|)) per 32x32 head block.
  y = sum_s (pw @ blockdiag(attn) @ diag(vdw_s) vw) @ shift_s(x2) + bias'
  as a 3x3 fp8 DoubleRow conv over an unpadded x2 plane; horizontal wrap
  reads are cancelled exactly by per-row correction columns (small matmuls
  vs. the x2 border columns), border bias columns as before.
"""
import numpy as np
import ml_dtypes

import concourse.bass as bass
import concourse.bacc as bacc
import concourse.mybir as mybir
import concourse.tile as tile
from concourse.bass_utils import run_bass_kernel_spmd
from concourse.masks import make_identity

BF = mybir.dt.bfloat16
F32 = mybir.dt.float32
FP8 = mybir.dt.float8e4
BF_NP = ml_dtypes.bfloat16
E4_NP = ml_dtypes.float8_e4m3
DR = mybir.MatmulPerfMode.DoubleRow

C = 128          # input channels (DIM)
D2 = 256         # qkv channels
HH = 128         # spatial H
WW = 128         # spatial W
PH = HH + 2
PITCH = 272      # fp8 x0/x1 plane row pitch: copy A at col 1, copy B at col 144
NPIX = HH * WW
SW = 8.0         # fp8 weight scale
SX = 16.0        # fp8 q/k input scale  (q~tilde units = SW*SX*q = 128 q)
SX2 = 32.0       # fp8 x2 scale (e4m3 max finite is 240)
SE = 32768.0     # fp8 E (folded attn+proj conv weight) scale
YSCALE = 1.0 / (SE * SX2)
NTILE = 32       # y-conv spatial tiles of 4 rows x 128 cols
X2N = 16768      # x2 plane length: 1 + 130*128 + pad, zero rows top/bottom
TAPS = [(dy, dx) for dy in (-1, 0, 1) for dx in (-1, 0, 1)]
# DoubleRow tap pairs: 3 vertical (dy=-1 with dy=0), 1 horizontal
# ((1,-1) with (1,0), address delta 1), 1 self-pair ((1,1) twice, halved)
PAIRS = [(0, 3), (1, 4), (2, 5), (6, 7), (8, 8)]
ADD = mybir.AluOpType.add
MULT = mybir.AluOpType.mult
AF = mybir.ActivationFunctionType

_CACHE = {}


def _win_pair_ap(xp, y, p):
    """lhsT AP for q/k conv row y, DoubleRow pair p: (128 cin, 2, 128 px).

    Ldweights dual-fp8 requires the pair step to be 16B-aligned; pair 3
    ((1,-1) with (1,0)) reads slot 1 from the B copy at col 144."""
    t0, t1 = PAIRS[p]
    dy0, dx0 = TAPS[t0]
    dy1, dx1 = TAPS[t1]
    off0 = (1 + y + dy0) * PITCH + 1 + dx0
    if p == 3:
        off1 = (1 + y + dy1) * PITCH + 144
    else:
        off1 = (1 + y + dy1) * PITCH + 1 + dx1
    return bass.AP(xp.tensor, xp.offset + off0,
                   [list(xp.ap[0]), [off1 - off0, 2], [1, 128]])


def _x2_pair_ap(xp, j, p):
    """rhs AP for y-conv tile j, DoubleRow pair p: (128 cin, 2, 512 px)."""
    t0, t1 = PAIRS[p]
    dy0, dx0 = TAPS[t0]
    dy1, dx1 = TAPS[t1]
    off0 = 1 + 128 * (4 * j + 1 + dy0) + dx0
    off1 = 1 + 128 * (4 * j + 1 + dy1) + dx1
    return bass.AP(xp.tensor, xp.offset + off0,
                   [list(xp.ap[0]), [off1 - off0, 2], [1, 512]])


def _qk_ap(qkT8, pc, col, ncols):
    """(128, 2, ncols) pair AP over rows (2pc, 2pc+1) of qkT8 at col offset."""
    return bass.AP(qkT8.tensor, qkT8.offset + 2 * pc * 512 + col,
                   [list(qkT8.ap[0]), [512, 2], [1, ncols]])


def _evac_out_ap(qkT8, j, col):
    """(128, 2, 2, 128) write AP: rows (2j, 2j+1) x col blocks {col, col+256}."""
    return bass.AP(qkT8.tensor, qkT8.offset + 2 * j * 512 + col,
                   [list(qkT8.ap[0]), [512, 2], [256, 2], [1, 128]])


def _gram(nc, qkT8, Gt, sks, ones8, pc):
    """DoubleRow Gram matmuls for pair-chunk pc: q^Tk and the ones-sums.

    The q^Tq / k^Tk self-blocks are no longer computed: norms come from
    Pool-squared copies summed by cheap ap-1 matmuls (see _sqsum)."""
    for g in range(2):
        qg = _qk_ap(qkT8, pc, 256 * g, 128)
        kg = _qk_ap(qkT8, pc, 256 * g + 128, 128)
        nc.tensor.matmul(Gt[:, g, :], qg, kg,
                         start=(pc == 0), stop=False, perf_mode=DR,
                         skip_group_check=True)
        nc.tensor.matmul(sks[:, 2 * g:2 * g + 1], qg, ones8,
                         start=(pc == 0), stop=(pc == 63),
                         perf_mode=DR, skip_group_check=True)
        nc.tensor.matmul(sks[:, 2 * g + 1:2 * g + 2], kg, ones8,
                         start=(pc == 0), stop=(pc == 63),
                         perf_mode=DR, skip_group_check=True)


def _sqsum(nc, sq8, sks, ones1b, col0, pc):
    """Accumulate per-channel sums of squares from the sq8 ring (bf16)."""
    for g in range(2):
        for r in range(2):
            nc.tensor.matmul(sks[:, col0 + g:col0 + g + 1],
                             sq8[:, pc % 8, r, g, :], ones1b,
                             start=(pc == 0 and r == 0), stop=(pc == 63 and r == 1),
                             skip_group_check=True)


def _build_nc():
    nc = bacc.Bacc(None, target_bir_lowering=False)

    x0d = nc.dram_tensor("x0", (C, PH, PITCH), FP8, kind="ExternalInput")
    x1d = nc.dram_tensor("x1", (C, PH, PITCH), FP8, kind="ExternalInput")
    x2d = nc.dram_tensor("x2", (C, X2N), FP8, kind="ExternalInput")
    aqd = nc.dram_tensor("aq8", (C, 10, D2), FP8, kind="ExternalInput")
    akd = nc.dram_tensor("ak8", (C, 10, D2), FP8, kind="ExternalInput")
    brd = nc.dram_tensor("brows", (1, 8, C), BF, kind="ExternalInput")
    bcd = nc.dram_tensor("bcor", (C, 8), F32, kind="ExternalInput")
    cvd = nc.dram_tensor("cv", (C, 9, 2, C), BF, kind="ExternalInput")
    pwtd = nc.dram_tensor("pwT", (C, 2, C), BF, kind="ExternalInput")
    bvd = nc.dram_tensor("bv", (C, 2, 9), BF, kind="ExternalInput")
    pbd = nc.dram_tensor("pbrow", (1, C), F32, kind="ExternalInput")
    e0d = nc.dram_tensor("e0row", (1, 9), F32, kind="ExternalInput")
    onesd = nc.dram_tensor("ones1", (1, C), BF, kind="ExternalInput")
    yd = nc.dram_tensor("y", (C, HH, WW), BF, kind="ExternalOutput")
    warmd = nc.dram_tensor("warmo", (1, 2), F32, kind="ExternalOutput")

    ROWCH = [0, 4, 12, 28, 46, 64, 82, 100, 116, 130]

    with tile.TileContext(nc) as tc:
        with (
            tc.tile_pool(name="consts", bufs=1) as consts,
            tc.tile_pool(name="xpad8", bufs=2) as xpad8,
            tc.tile_pool(name="x2pool", bufs=1) as x2pool,
            tc.tile_pool(name="qkp", bufs=1) as qkp,
            tc.tile_pool(name="small", bufs=1) as small,
            tc.tile_pool(name="ysb", bufs=6) as ysb,
            tc.tile_pool(name="cpsum", bufs=3, space="PSUM") as cpsum,
            tc.tile_pool(name="gpsum", bufs=1, space="PSUM") as gpsum,
            tc.tile_pool(name="mpsum", bufs=3, space="PSUM") as mpsum,
        ):
            # ---- input + const DMAs, latency-ordered ----
            x0p = xpad8.tile([C, PH, PITCH], FP8, tag="xp8")
            nc.sync.dma_start(out=x0p[:, ROWCH[0]:ROWCH[1], :],
                              in_=x0d[:, ROWCH[0]:ROWCH[1], :])
            aq8 = consts.tile([C, 10, D2], FP8)
            nc.sync.dma_start(out=aq8, in_=aqd[:, :, :])
            for ci in range(1, 9):
                nc.sync.dma_start(out=x0p[:, ROWCH[ci]:ROWCH[ci + 1], :],
                                  in_=x0d[:, ROWCH[ci]:ROWCH[ci + 1], :])
            ak8 = consts.tile([C, 10, D2], FP8)
            nc.sync.dma_start(out=ak8, in_=akd[:, :, :])
            x1p = xpad8.tile([C, PH, PITCH], FP8, tag="xp8")
            for ci in range(9):
                nc.sync.dma_start(out=x1p[:, ROWCH[ci]:ROWCH[ci + 1], :],
                                  in_=x1d[:, ROWCH[ci]:ROWCH[ci + 1], :])
            x2p = x2pool.tile([C, X2N], FP8, tag="x2p")
            for ci in range(4):
                nc.sync.dma_start(out=x2p[:, 4192 * ci:4192 * (ci + 1)],
                                  in_=x2d[:, 4192 * ci:4192 * (ci + 1)])
            brows = consts.tile([1, 8, C], BF)
            nc.sync.dma_start(out=brows, in_=brd[:, :, :])
            bcor = consts.tile([C, 8], F32)
            nc.sync.dma_start(out=bcor, in_=bcd[:, :])
            cv = consts.tile([C, 9, 2, C], BF)
            nc.sync.dma_start(out=cv, in_=cvd[:, :, :, :])
            pwt = consts.tile([C, 2, C], BF)
            nc.sync.dma_start(out=pwt, in_=pwtd[:, :, :])
            bv = consts.tile([C, 2, 9], BF)
            nc.sync.dma_start(out=bv, in_=bvd[:, :, :])
            pbrow = consts.tile([1, C], F32)
            nc.sync.dma_start(out=pbrow, in_=pbd[:, :])
            e0row = consts.tile([1, 9], F32)
            nc.sync.dma_start(out=e0row, in_=e0d[:, :])
            ones1 = consts.tile([1, C], BF)
            nc.sync.dma_start(out=ones1, in_=onesd[:, :])
            identf = consts.tile([128, 128], F32)
            make_identity(nc, identf)
            ones8t = consts.tile([128, 2, 16], FP8)
            nc.vector.memset(ones8t.rearrange("p a b -> p (a b)"), 1.0)
            ones8 = ones8t[:, :, 0:1]

            # scratch for activation-table preloads (see below)
            warm = small.tile([1, 2], F32)
            nc.vector.memset(warm, 1.0)
            warm2 = small.tile([1, 2], F32)

            # PE p-state warm-up during the initial DMA wait
            idrep = bass.AP(identf.tensor, identf.offset,
                            [list(identf.ap[0]), [0, 4], [1, 128]])
            dwp = mpsum.tile([128, 512], F32, tag="mp")
            nc.tensor.matmul(dwp, identf, idrep, start=True, stop=True)
            nc.vector.tensor_copy(warm2[:, 1:2], dwp[0:1, 0:1])

            # ---- big SBUF tensors ----
            qkT8 = qkp.tile([128, 128, 512], FP8)   # [px, row, q0|k0|q1|k1]

            # ---- small tiles ----
            dq2 = small.tile([128, 2], F32)
            dk2 = small.tile([128, 2], F32)
            qinv = small.tile([128, 2], F32)
            kinv = small.tile([128, 2], F32)
            kir = small.tile([1, 2, C], BF)
            kb = small.tile([128, 2, C], F32)
            lblk = small.tile([128, 2, 32], F32)
            ablk = small.tile([128, 2, 32], F32)
            rs = small.tile([128, 2], F32)
            rr = small.tile([128, 2], F32)
            attnBD = small.tile([128, 2, D2], BF)
            pat = small.tile([128, 2, C], BF)
            eall8 = small.tile([128, 5, 2, C], FP8)   # y-conv DR pair weights
            eallB = small.tile([128, 2, 3, C], BF)    # dx=-1 / dx=+1 taps
            coly = small.tile([128, 9], F32)
            u0 = small.tile([128, 131], BF)           # u0[j] = plane[1+128j]
            u127 = small.tile([128, 130], BF)         # u127[m] = plane[128m]
            corrT = small.tile([128, 128, 2], F32)    # wrap+bias col corrections

            evac_fns = [lambda o, i: nc.vector.tensor_copy(o, i),
                        lambda o, i: nc.scalar.copy(o, i)]
            Gt = gpsum.tile([128, 2, C], F32, tag="G")
            # sks cols: [Sq0, Sk0, Sq1, Sk1, Q2_0, Q2_1, K2_0, K2_1]
            sks = gpsum.tile([128, 8], F32, tag="sk")
            sq8 = qkp.tile([128, 8, 2, 2, 128], BF)   # squared-pair ring
            ones128 = consts.tile([128, 1], BF)
            nc.vector.memset(ones128, 1.0)

            # ---- q/k convs (fp8 DoubleRow, qT-direct layout) + lagged gram ----
            for conv, (wts, xp, colbase) in enumerate(
                    ((aq8, x0p, 0), (ak8, x1p, 128))):
                for j in range(64):
                    # de-bunch the gram tail: pcs 61/62 are ready before the
                    # j=63 conv matmuls, emit them first
                    if conv == 1 and j == 63:
                        for pc in (61, 62):
                            _gram(nc, qkT8, Gt, sks, ones8, pc)
                    acc = cpsum.tile([128, 512], F32, tag="cacc")
                    for r in range(2):
                        y = 2 * j + r
                        o = acc[:, 256 * r:256 * r + 256]
                        for p in range(5):
                            nc.tensor.matmul(o, _win_pair_ap(xp, y, p),
                                             wts[:, 2 * p:2 * p + 2, :],
                                             start=(p == 0), stop=(p == 4),
                                             perf_mode=DR)
                    src = acc.rearrange("p (r b i) -> p r b i", r=2, b=2)
                    evac_fns[j % 2](_evac_out_ap(qkT8, j, colbase), src)
                    # square the evacuated pair on the idle Pool engine
                    sqsrc = _evac_out_ap(qkT8, j, colbase)
                    nc.gpsimd.tensor_mul(sq8[:, j % 8, :, :, :], sqsrc, sqsrc)
                    # lagged sum-of-squares (norms) on cheap ap-1 matmuls
                    if j >= 3:
                        _sqsum(nc, sq8, sks, ones128, 4 + 2 * conv, j - 3)
                    if conv == 1 and j == 2:
                        for pc in (61, 62, 63):
                            _sqsum(nc, sq8, sks, ones128, 4, pc)

                    # gram for pair-chunk pc (lagged) during the k conv
                    if conv == 1:
                        pcs = [j - 2] if j >= 2 else []
                        if j == 63:
                            pcs = [63]
                        for pc in pcs:
                            _gram(nc, qkT8, Gt, sks, ones8, pc)


            # preload the ln/exp table set (one set holds BOTH Ln and Exp;
            # rsqrt is computed as exp(-0.5 ln x) so the whole attn phase
            # runs on a single act-table set with zero switches). The
            # scheduler hoists this no-dep op to t~0, which is exactly right.
            nc.scalar.activation(out=warm2[:, 0:1], in_=warm[:, 0:1],
                                 func=AF.Ln)
            nc.sync.dma_start(out=warmd[:, :], in_=warm2)

            for pc in (61, 62, 63):
                _sqsum(nc, sq8, sks, ones128, 6, pc)

            # ---- norms: |q|^2 = Q2 + 2 bq Sq + N bq^2 (column ops only) ----
            SqAP = bass.AP(sks.tensor, sks.offset, [list(sks.ap[0]), [2, 2]])
            SkAP = bass.AP(sks.tensor, sks.offset + 1,
                           [list(sks.ap[0]), [2, 2]])
            tq2 = small.tile([128, 2], F32)
            tk2 = small.tile([128, 2], F32)
            nc.vector.tensor_tensor(out=tq2, in0=bcor[:, 0:2], in1=SqAP, op=MULT)
            nc.vector.tensor_tensor(out=dq2, in0=sks[:, 4:6], in1=tq2, op=ADD)
            nc.vector.tensor_tensor(out=dq2, in0=dq2, in1=bcor[:, 2:4], op=ADD)
            nc.vector.tensor_tensor(out=tk2, in0=bcor[:, 4:6], in1=SkAP, op=MULT)
            nc.vector.tensor_tensor(out=dk2, in0=sks[:, 6:8], in1=tk2, op=ADD)
            nc.vector.tensor_tensor(out=dk2, in0=dk2, in1=bcor[:, 6:8], op=ADD)
            lnq = small.tile([128, 2], F32)
            lnk = small.tile([128, 2], F32)
            nc.scalar.activation(out=lnq, in_=dq2, func=AF.Ln)
            nc.scalar.activation(out=qinv, in_=lnq, func=AF.Exp, scale=-0.5)
            nc.scalar.activation(out=lnk, in_=dk2, func=AF.Ln, scale=16.0)
            nc.scalar.activation(out=kinv, in_=lnk, func=AF.Exp, scale=-0.5)

            # ---- bias corrections as rank-1 closes (q^Tk block only) ----
            # brows host rows: [bq0, bq1, bk0, bk1, -, -, Nbk0, Nbk1]
            scols = small.tile([128, 4], F32)
            nc.vector.tensor_copy(scols, sks[:, 0:4])
            rws = small.tile([1, 8, C], BF)
            for g in range(2):
                tpq = mpsum.tile([1, C], F32, tag="mp")
                nc.tensor.transpose(tpq, scols[:, 2 * g:2 * g + 1], identf)
                nc.scalar.copy(rws[:, 0 + g, :], tpq)
                tpk = mpsum.tile([1, C], F32, tag="mp")
                nc.tensor.transpose(tpk, scols[:, 2 * g + 1:2 * g + 2], identf)
                nc.vector.tensor_tensor(out=rws[:, 6 + g, :], in0=tpk,
                                        in1=brows[:, 6 + g, :], op=ADD)
            for g in range(2):
                nc.tensor.matmul(Gt[:, g, :], brows[:, 0 + g, :],
                                 rws[:, 6 + g, :], start=False, stop=False,
                                 skip_group_check=True)
                nc.tensor.matmul(Gt[:, g, :], rws[:, 0 + g, :],
                                 brows[:, 2 + g, :], start=False, stop=True,
                                 skip_group_check=True)


            # ---- softmax per 32x32 head block -> attnBD ----
            nc.vector.memset(attnBD.rearrange("p a b -> p (a b)"), 0.0)
            for g in range(2):
                kt = mpsum.tile([1, C], F32, tag="mp")
                nc.tensor.transpose(kt, kinv[:, g:g + 1], identf)
                evac_fns[g](kir[:, g, :], kt)
            kbp = mpsum.tile([128, 2, C], F32, tag="mp")
            nc.tensor.matmul(kbp.rearrange("p a b -> p (a b)"), ones1,
                             kir.rearrange("p a b -> p (a b)"),
                             start=True, stop=True)
            nc.vector.tensor_copy(kb[:, 0, :], kbp[:, 0, :])
            nc.scalar.copy(kb[:, 1, :], kbp[:, 1, :])
            for g in range(2):
                for b in range(4):
                    p0 = 32 * b
                    nc.vector.tensor_tensor(
                        out=lblk[p0:p0 + 32, g, :],
                        in0=Gt[p0:p0 + 32, g, p0:p0 + 32],
                        in1=kb[p0:p0 + 32, g, p0:p0 + 32],
                        op=MULT)
                nc.scalar.activation(
                    out=ablk[:, g, :], in_=lblk[:, g, :],
                    func=AF.Exp, scale=qinv[:, g:g + 1],
                    accum_out=rs[:, g:g + 1])
                nc.vector.reciprocal(out=rr[:, g:g + 1], in_=rs[:, g:g + 1])
                for b in range(4):
                    p0 = 32 * b
                    nc.vector.tensor_scalar(
                        out=attnBD[p0:p0 + 32, g, 128 * g + p0:128 * g + p0 + 32],
                        in0=ablk[p0:p0 + 32, g, :],
                        scalar1=rr[p0:p0 + 32, g:g + 1], scalar2=None, op0=MULT)

            # ---- PA^T = attnBD^T @ pw^T ----
            patp = mpsum.tile([128, 2, C], F32, tag="mp")
            for mc in range(2):
                for kc in range(2):
                    nc.tensor.matmul(patp[:, mc, :],
                                     attnBD[:, kc, 128 * mc:128 * mc + 128],
                                     pwt[:, kc, :], start=(kc == 0), stop=(kc == 1))
            nc.vector.tensor_copy(pat[:, 0, :], patp[:, 0, :])
            nc.scalar.copy(pat[:, 1, :], patp[:, 1, :])

            # ---- E_s^T = C_s^T @ PA^T  (y-conv weights, SE-scaled via cv) ----
            # fp8 pair-slot layout + bf16 copies of the dx=+-1 taps for the
            # wrap corrections
            SLOT = {}
            for p, (t0, t1) in enumerate(PAIRS):
                SLOT.setdefault(t0, []).append((p, 0))
                SLOT.setdefault(t1, []).append((p, 1))
            SLOT[8] = [(4, 0), (4, 1)]
            BSLOT = {0: (0, 0), 3: (0, 1), 6: (0, 2),
                     2: (1, 0), 5: (1, 1), 8: (1, 2)}
            for si, s in enumerate((0, 3, 1, 4, 2, 5, 6, 7, 8)):
                ep = mpsum.tile([128, C], F32, tag="mp")
                for kc in range(2):
                    nc.tensor.matmul(ep, cv[:, s, kc, :], pat[:, kc, :],
                                     start=(kc == 0), stop=(kc == 1))
                cp_a = evac_fns[si % 2]
                cp_b = evac_fns[(si + 1) % 2]
                if s == 8:  # self-pair: both slots at half weight
                    for p, sl in SLOT[8]:
                        nc.vector.tensor_scalar(out=eall8[:, p, sl, :], in0=ep,
                                                scalar1=0.5, scalar2=None,
                                                op0=MULT)
                else:
                    for p, sl in SLOT[s]:
                        cp_a(eall8[:, p, sl, :], ep)
                if s in BSLOT:
                    side, di = BSLOT[s]
                    cp_b(eallB[:, side, di, :], ep)

            # ---- bias columns (interior + border deltas) ----
            wp = mpsum.tile([128, 9], F32, tag="mp")
            for kc in range(2):
                nc.tensor.matmul(wp, pat[:, kc, :], bv[:, kc, :],
                                 start=(kc == 0), stop=False)
            nc.tensor.matmul(wp, pbrow, e0row, start=False, stop=True)
            nc.vector.tensor_copy(coly, wp)

            # ---- wrap-correction columns ----
            # u0[c, j] = x2plane[c, 1+128j] (img col 0), u127[c, i] =
            # x2plane[c, 128(i+1)] (img col 127); both SX2-scaled, zero-padded.
            nc.vector.tensor_copy(
                u0, bass.AP(x2p.tensor, x2p.offset + 1,
                            [list(x2p.ap[0]), [128, 131]]))
            nc.vector.tensor_copy(
                u127, bass.AP(x2p.tensor, x2p.offset,
                              [list(x2p.ap[0]), [128, 130]]))
            c0p = mpsum.tile([128, C], F32, tag="mp")
            for di, dy in enumerate((-1, 0, 1)):
                nc.tensor.matmul(c0p, eallB[:, 0, di, :],
                                 u127[:, dy + 1:dy + 129],
                                 start=(di == 0), stop=(di == 2))
            nc.vector.tensor_scalar(out=corrT[:, :, 0:1], in0=c0p,
                                    scalar1=-YSCALE, scalar2=coly[:, 3:4],
                                    op0=MULT, op1=ADD)
            c1p = mpsum.tile([128, C], F32, tag="mp")
            for di, dy in enumerate((-1, 0, 1)):
                nc.tensor.matmul(c1p, eallB[:, 1, di, :],
                                 u0[:, dy + 2:dy + 130],
                                 start=(di == 0), stop=(di == 2))
            nc.vector.tensor_scalar(out=corrT[:, :, 1:2], in0=c1p,
                                    scalar1=-YSCALE, scalar2=coly[:, 4:5],
                                    op0=MULT, op1=ADD)

            # ---- y conv (fp8 DoubleRow) ----
            yt2 = None
            for j in range(NTILE):
                acc = cpsum.tile([128, 512], F32, tag="cacc")
                for p in range(5):
                    nc.tensor.matmul(acc, eall8[:, p, :, :], _x2_pair_ap(x2p, j, p),
                                     start=(p == 0), stop=(p == 4), perf_mode=DR)
                if j % 2 == 0:
                    yt2 = ysb.tile([128, 8, 128], BF, tag="yt")
                yt = yt2[:, 4 * (j % 2):4 * (j % 2) + 4, :]
                ytf = yt.rearrange("p a b -> p (a b)")
                if j % 2 == 0:
                    nc.vector.tensor_scalar(out=ytf, in0=acc, scalar1=YSCALE,
                                            scalar2=coly[:, 0:1], op0=MULT,
                                            op1=ADD)
                else:
                    nc.scalar.activation(out=ytf, in_=acc, func=AF.Identity,
                                         bias=coly[:, 0:1], scale=YSCALE)
                # border columns: wrap correction + col bias deltas, applied
                # once per double-tile (per-tile for the final pair, which is
                # DMA'd as two singles to shorten the tail)
                if j % 2 == 1 and j < 30:
                    r0 = 4 * (j - 1)
                    nc.vector.tensor_tensor(out=yt2[:, :, 0:1],
                                            in0=yt2[:, :, 0:1],
                                            in1=corrT[:, r0:r0 + 8, 0:1],
                                            op=ADD)
                    nc.vector.tensor_tensor(out=yt2[:, :, 127:128],
                                            in0=yt2[:, :, 127:128],
                                            in1=corrT[:, r0:r0 + 8, 1:2],
                                            op=ADD)
                elif j >= 30:
                    nc.vector.tensor_tensor(out=yt[:, :, 0:1],
                                            in0=yt[:, :, 0:1],
                                            in1=corrT[:, 4 * j:4 * j + 4, 0:1],
                                            op=ADD)
                    nc.vector.tensor_tensor(out=yt[:, :, 127:128],
                                            in0=yt[:, :, 127:128],
                                            in1=corrT[:, 4 * j:4 * j + 4, 1:2],
                                            op=ADD)
                cs = lambda i: coly[:, i:i + 1]
                if j == 0:
                    nc.vector.tensor_scalar(out=yt[:, 0, :], in0=yt[:, 0, :],
                                            scalar1=cs(1), scalar2=None, op0=ADD)
                    nc.vector.tensor_scalar(out=yt[:, 0, 0:1], in0=yt[:, 0, 0:1],
                                            scalar1=cs(5), scalar2=None, op0=ADD)
                    nc.vector.tensor_scalar(out=yt[:, 0, 127:128],
                                            in0=yt[:, 0, 127:128],
                                            scalar1=cs(6), scalar2=None, op0=ADD)
                if j == NTILE - 1:
                    nc.vector.tensor_scalar(out=yt[:, 3, :], in0=yt[:, 3, :],
                                            scalar1=cs(2), scalar2=None, op0=ADD)
                    nc.vector.tensor_scalar(out=yt[:, 3, 0:1], in0=yt[:, 3, 0:1],
                                            scalar1=cs(7), scalar2=None, op0=ADD)
                    nc.vector.tensor_scalar(out=yt[:, 3, 127:128],
                                            in0=yt[:, 3, 127:128],
                                            scalar1=cs(8), scalar2=None, op0=ADD)
                if j >= 30:
                    nc.sync.dma_start(out=yd[:, 4 * j:4 * j + 4, :], in_=yt)
                elif j % 2 == 1:
                    nc.sync.dma_start(out=yd[:, 4 * (j - 1):4 * (j - 1) + 8, :],
                                      in_=yt2)

    nc.compile()
    return nc


def _host_consts(qw, qb, kw, kb, vw, vb, qdw, qdb, kdw, kdb, vdw, vdb, pw, pb):
    """Fold all static weights into the forms the kernel consumes."""
    qw2, kw2, vw2, pw2 = [w[:, :, 0, 0].astype(np.float64) for w in (qw, kw, vw, pw)]
    qd, kd, vd = [w[:, 0].astype(np.float64) for w in (qdw, kdw, vdw)]

    def conv_w8(d, w2):
        # (C, 10, D2) fp8 rhs, slots (2p, 2p+1) = the DoubleRow pair p
        a = np.zeros((C, 10, D2), np.float32)
        for p, (t0, t1) in enumerate(PAIRS):
            h = 0.5 if t0 == t1 else 1.0
            for s_, t in ((0, t0), (1, t1)):
                dy, dx = TAPS[t]
                a[:, 2 * p + s_, :] = (h * SW * d[:, dy + 1, dx + 1][:, None] * w2).T
        return a.astype(E4_NP)

    def bias_cols(b1, db, d):
        cols = np.stack([
            db + b1 * d.sum((-2, -1)),
            -b1 * d[:, 0, :].sum(-1), -b1 * d[:, 2, :].sum(-1),
            -b1 * d[:, :, 0].sum(-1), -b1 * d[:, :, 2].sum(-1),
            b1 * d[:, 0, 0], b1 * d[:, 0, 2], b1 * d[:, 2, 0], b1 * d[:, 2, 2],
        ], axis=-1)  # (256, 9)
        return cols.reshape(2, 128, 9).transpose(1, 0, 2).astype(BF_NP)

    # interior bias in q~tilde units (x SW*SX)
    bq = (SW * SX) * (qdb.astype(np.float64) + qb.astype(np.float64) * qd.sum((-2, -1)))
    bk = (SW * SX) * (kdb.astype(np.float64) + kb.astype(np.float64) * kd.sum((-2, -1)))
    brows = np.zeros((1, 8, C), np.float64)
    bcor = np.zeros((C, 8), np.float64)
    for g in range(2):
        brows[0, 0 + g] = bq[128 * g:128 * g + 128]
        brows[0, 2 + g] = bk[128 * g:128 * g + 128]
        brows[0, 4 + g] = NPIX * brows[0, 0 + g]
        brows[0, 6 + g] = NPIX * brows[0, 2 + g]
        bcor[:, 0 + g] = 2.0 * brows[0, 0 + g]
        bcor[:, 2 + g] = NPIX * brows[0, 0 + g] ** 2
        bcor[:, 4 + g] = 2.0 * brows[0, 2 + g]
        bcor[:, 6 + g] = NPIX * brows[0, 2 + g] ** 2

    cvf = np.stack([(vd[:, dy + 1, dx + 1][:, None] * vw2)
                    for (dy, dx) in TAPS])             # (9, 256, 128)
    cvf = (SE * cvf).reshape(9, 2, 128, 128).transpose(2, 0, 1, 3).astype(BF_NP)
    pwT = pw2.T.reshape(2, 128, 128).transpose(1, 0, 2).astype(BF_NP)
    e0 = np.zeros((1, 9), np.float32)
    e0[0, 0] = 1.0
    return {
        "aq8": conv_w8(qd, qw2), "ak8": conv_w8(kd, kw2),
        "brows": brows.astype(BF_NP),
        "bcor": bcor.astype(np.float32),
        "cv": cvf, "pwT": pwT,
        "bv": bias_cols(vb.astype(np.float64), vdb.astype(np.float64), vd),
        "pbrow": pb.reshape(1, C).astype(np.float32),
        "e0row": e0,
        "ones1": np.ones((1, C), BF_NP),
    }


def _host_inputs(inputs):
    consts = _host_consts(**{k: np.asarray(inputs[k]) for k in
                             ("qw", "qb", "kw", "kb", "vw", "vb", "qdw", "qdb",
                              "kdw", "kdb", "vdw", "vdb", "pw", "pb")})
    n = np.asarray(inputs["x0"]).shape[0]
    x0 = np.zeros((n, C, PH, PITCH), np.float32)
    x0[:, :, 1:1 + HH, 1:1 + WW] = np.asarray(inputs["x0"]) * SX
    x0[:, :, 1:1 + HH, 144:144 + WW] = x0[:, :, 1:1 + HH, 1:1 + WW]
    x1 = np.zeros((n, C, PH, PITCH), np.float32)
    x1[:, :, 1:1 + HH, 1:1 + WW] = np.asarray(inputs["x1"]) * SX
    x1[:, :, 1:1 + HH, 144:144 + WW] = x1[:, :, 1:1 + HH, 1:1 + WW]
    x2 = np.zeros((n, C, X2N), np.float32)
    x2[:, :, 129:129 + NPIX] = (np.asarray(inputs["x2"]) * SX2).reshape(n, C, NPIX)
    return consts, x0.astype(E4_NP), x1.astype(E4_NP), x2.astype(E4_NP)


def kernel(**inputs):
    if "nc" not in _CACHE:
        _CACHE["nc"] = _build_nc()
    nc = _CACHE["nc"]
    consts, x0, x1, x2 = _host_inputs(inputs)
    n_cores = x0.shape[0]
    in_maps = [dict(consts, x0=x0[i], x1=x1[i], x2=x2[i]) for i in range(n_cores)]
    res = run_bass_kernel_spmd(nc, in_maps, list(range(n_cores)))
    _CACHE["last_res"] = res
    return np.stack([np.asarray(r["y"]) for r in res.results]).astype(np.float32)


def kernel_sim(**inputs):
    """CoreSim validation path: run sample 0 only through the simulator."""
    from concourse.bass_interp import CoreSim

    if "nc" not in _CACHE:
        _CACHE["nc"] = _build_nc()
    nc = _CACHE["nc"]
    consts, x0, x1, x2 = _host_inputs(inputs)
    sim = CoreSim(nc)
    for name, arr in consts.items():
        sim.tensor(name)[:] = arr
    sim.tensor("x0")[:] = x0[0]
    sim.tensor("x1")[:] = x1[0]
    sim.tensor("x2")[:] = x2[0]
    sim.simulate()
    _CACHE["sim"] = sim
    return np.array(sim.tensor("y"))[None].astype(np.float32)
